# revision 5
# baseline (speedup 1.0000x reference)
"""Trainium2 Bass kernel for a GRU (B=64, T=512, D=256, H=512), 8-core data-parallel.

Strategy
--------
- Data-parallel over batch: 8 samples per NeuronCore, 8 cores.
- Everything on-chip lives in a transposed [H(partition-chunked), ..., B] layout so
  the recurrent matmuls (U stationary, h streaming) and the elementwise gate math
  share a layout: no per-step transposes.
- Phase 1 (per core): xz/xr/xh = x @ W* + b* computed as stream-bound matmuls into
  DRAM scratch, output layout [128p, 4m, T, 8b] (h = m*128+p), bf16.
- Phase 2: 512 sequential GRU steps. Per step and gate, PSUM accumulates
  identity-matmul(xg_t) + sum_k U[k,m]^T @ h[k]; ScalarE applies sigmoid/tanh,
  VectorE does the blend h' = (1-z)*h~ + z*h in fp32; a bf16 copy of h feeds the
  next step's matmuls. U matrices are bf16 (fast weight load), state is fp32.
- Host side: shards batch, pre-transposes x, casts U to bf16, and re-assembles the
  [B, T, H] output + final hT from the per-core [128, T, 32] outputs.
"""

import numpy as np
import ml_dtypes
from contextlib import ExitStack

import concourse.bass as bass
import concourse.bacc as bacc
import concourse.tile as tile
import concourse.mybir as mybir
from concourse.bass_utils import run_bass_kernel_spmd

B, T, D, H = 64, 512, 256, 512
NCORES = 8
BL = B // NCORES            # local batch per core = 8
KD = D // 128               # 2 contraction chunks for x @ W
KH = H // 128               # 4 chunks of the hidden dim
FB = KH * BL                # 32 free columns of a [h-chunked, batch] tile
TPRE = 8                    # precompute tiles over the (t, b) column axis
TC = 32                     # steps per recurrence chunk (xg staging + hist drain)

FP32 = mybir.dt.float32
BF16 = mybir.dt.bfloat16
AF = mybir.ActivationFunctionType


def _build(n_steps: int = T):
    """Build the single-core Bass program (SPMD across 8 cores)."""
    assert n_steps % TC == 0
    nc = bacc.Bacc("TRN2", target_bir_lowering=False, debug=False, num_devices=NCORES)

    xT = nc.dram_tensor("xT", [D, n_steps * BL], FP32, kind="ExternalInput")
    Ws = {g: nc.dram_tensor(f"W{g}", [D, H], FP32, kind="ExternalInput") for g in "zrh"}
    Us = {g: nc.dram_tensor(f"U{g}", [H, H], BF16, kind="ExternalInput") for g in "zrh"}
    bs = {g: nc.dram_tensor(f"b{g}", [128, KH], FP32, kind="ExternalInput") for g in "zrh"}
    ident = nc.dram_tensor("ident", [128, 128], BF16, kind="ExternalInput")
    yT = nc.dram_tensor("yT", [128, n_steps, FB], FP32, kind="ExternalOutput")

    pre_cols = n_steps * BL // TPRE         # columns per precompute tile
    pre_steps = n_steps // TPRE             # time steps per precompute tile

    with tile.TileContext(nc) as tc, ExitStack() as ctx:
        wpool = ctx.enter_context(tc.tile_pool(name="wpool", bufs=1))
        dram = ctx.enter_context(tc.tile_pool(name="dram", bufs=1, space="DRAM"))

        # --- load weights -------------------------------------------------
        U_sb, W_sb, b_sb = {}, {}, {}
        for g in "zrh":
            u_t = wpool.tile([128, KH, H], BF16, tag=f"U{g}", name=f"U{g}_sb")
            nc.sync.dma_start(u_t[:], Us[g].rearrange("(k p) c -> p k c", p=128))
            U_sb[g] = u_t
            w_t = wpool.tile([128, KD, H], FP32, tag=f"W{g}", name=f"W{g}_sb")
            nc.sync.dma_start(w_t[:], Ws[g].rearrange("(k p) c -> p k c", p=128))
            W_sb[g] = w_t
            bias_t = wpool.tile([128, KH], FP32, tag=f"b{g}", name=f"b{g}_sb")
            nc.sync.dma_start(bias_t[:], bs[g][:])
            b_sb[g] = bias_t
        I_sb = wpool.tile([128, 128], BF16, tag="ident", name="I_sb")
        nc.sync.dma_start(I_sb[:], ident[:])

        # DRAM scratch for the precomputed input projections, [p, m, t, b] bf16
        xg_dr = {
            g: dram.tile([128, KH, n_steps, BL], BF16, tag=f"xgd{g}", name=f"xgd{g}")
            for g in "zrh"
        }

        # --- phase 1: xg = x @ Wg + bg ------------------------------------
        with (
            tc.tile_pool(name="xtp", bufs=2) as xt_pool,
            tc.tile_pool(name="preps", bufs=2, space="PSUM") as pre_ps,
            tc.tile_pool(name="preout", bufs=3) as pre_out,
        ):
            xTr = xT.rearrange("(k p) n -> p k n", p=128)
            for tt in range(TPRE):
                xt = xt_pool.tile([128, KD, pre_cols], FP32, tag="xt", name="xt")
                nc.sync.dma_start(
                    xt[:], xTr[:, :, tt * pre_cols : (tt + 1) * pre_cols]
                )
                for g in "zrh":
                    for m in range(KH):
                        ps = pre_ps.tile([128, pre_cols], FP32, tag="ps", name="ps")
                        for k in range(KD):
                            nc.tensor.matmul(
                                ps[:],
                                W_sb[g][:, k, m * 128 : (m + 1) * 128],
                                xt[:, k, :],
                                start=(k == 0),
                                stop=(k == KD - 1),
                            )
                        ob = pre_out.tile([128, pre_cols], BF16, tag="ob", name="ob")
                        nc.scalar.activation(
                            ob[:], ps[:], AF.Identity, bias=b_sb[g][:, m : m + 1]
                        )
                        nc.sync.dma_start(
                            xg_dr[g][:, m, tt * pre_steps : (tt + 1) * pre_steps, :],
                            ob[:],
                        )

        # --- phase 2: recurrence ------------------------------------------
        with (
            tc.tile_pool(name="hstate", bufs=3) as hpool,
            tc.tile_pool(name="gps", bufs=2, space="PSUM") as gates_ps,
            tc.tile_pool(name="ew", bufs=2) as ew,
            tc.tile_pool(name="xgs", bufs=2) as xg_pool,
            tc.tile_pool(name="hist", bufs=2) as hist_pool,
        ):
            h_bf = hpool.tile([128, KH, BL], BF16, tag="hbf", name="h_bf")
            nc.vector.memset(h_bf[:], 0.0)
            h_fp_prev = None  # fp32 h of the previous step (a hist slice)

            for c in range(n_steps // TC):
                xg_sb = {}
                for g in "zrh":
                    xg_t = xg_pool.tile(
                        [128, KH, TC, BL], BF16, tag=f"xg{g}", name=f"xg{g}_sb"
                    )
                    nc.sync.dma_start(xg_t[:], xg_dr[g][:, :, c * TC : (c + 1) * TC, :])
                    xg_sb[g] = xg_t
                hist = hist_pool.tile([128, TC, FB], FP32, tag="hist", name="hist")

                for tl in range(TC):
                    ps = {
                        g: gates_ps.tile(
                            [128, KH, BL], FP32, tag=f"ps{g}", name=f"ps{g}"
                        )
                        for g in "zrh"
                    }
                    # accumulate xg_t into PSUM via an identity matmul
                    for g in "zrh":
                        nc.tensor.matmul(
                            ps[g][:], I_sb[:], xg_sb[g][:, :, tl, :],
                            start=True, stop=False,
                        )
                    # r gate first (its output gates the h~ matmul)
                    for k in range(KH):
                        for m in range(KH):
                            nc.tensor.matmul(
                                ps["r"][:, m, :],
                                U_sb["r"][:, k, m * 128 : (m + 1) * 128],
                                h_bf[:, k, :],
                                start=False,
                                stop=(k == KH - 1 and m == KH - 1),
                            )
                    r_bf = ew.tile([128, KH, BL], BF16, tag="r", name="r_bf")
                    nc.scalar.activation(r_bf[:], ps["r"][:], AF.Sigmoid)
                    rh = ew.tile([128, KH, BL], BF16, tag="rh", name="rh")
                    nc.vector.tensor_mul(rh[:], r_bf[:], h_bf[:])

                    # z gate (keeps PE busy during sigmoid(r) / r*h)
                    for k in range(KH):
                        for m in range(KH):
                            nc.tensor.matmul(
                                ps["z"][:, m, :],
                                U_sb["z"][:, k, m * 128 : (m + 1) * 128],
                                h_bf[:, k, :],
                                start=False,
                                stop=(k == KH - 1 and m == KH - 1),
                            )
                    # candidate gate uses r*h
                    for k in range(KH):
                        for m in range(KH):
                            nc.tensor.matmul(
                                ps["h"][:, m, :],
                                U_sb["h"][:, k, m * 128 : (m + 1) * 128],
                                rh[:, k, :],
                                start=False,
                                stop=(k == KH - 1 and m == KH - 1),
                            )

                    z = ew.tile([128, KH, BL], FP32, tag="z", name="z_t")
                    nc.scalar.activation(z[:], ps["z"][:], AF.Sigmoid)
                    zc = ew.tile([128, KH, BL], FP32, tag="zc", name="zc_t")
                    nc.scalar.activation(zc[:], ps["z"][:], AF.Sigmoid, scale=-1.0)
                    ht = ew.tile([128, KH, BL], FP32, tag="ht", name="ht_t")
                    nc.scalar.activation(ht[:], ps["h"][:], AF.Tanh)

                    h_new = hist[:, tl, :].rearrange("p (m b) -> p m b", m=KH)
                    a = ew.tile([128, KH, BL], FP32, tag="a", name="a_t")
                    nc.vector.tensor_mul(a[:], zc[:], ht[:])
                    if h_fp_prev is None:
                        # t == 0: h_prev = 0 so h' = (1-z)*h~
                        nc.vector.tensor_copy(h_new, a[:])
                    else:
                        zh = ew.tile([128, KH, BL], FP32, tag="zh", name="zh_t")
                        nc.vector.tensor_mul(zh[:], z[:], h_fp_prev)
                        nc.vector.tensor_add(h_new, a[:], zh[:])
                    h_bf = hpool.tile([128, KH, BL], BF16, tag="hbf", name="h_bf")
                    nc.vector.tensor_copy(h_bf[:], h_new)
                    h_fp_prev = h_new

                nc.sync.dma_start(yT[:, c * TC : (c + 1) * TC, :], hist[:])

    nc.compile()
    return nc


_NC_CACHE = {}


def _get_nc(n_steps: int = T):
    if n_steps not in _NC_CACHE:
        _NC_CACHE[n_steps] = _build(n_steps)
    return _NC_CACHE[n_steps]


def _in_maps(x, Wz, Wr, Wh, Uz, Ur, Uh, bz, br, bh, n_steps: int = T):
    bf = ml_dtypes.bfloat16
    common = {
        "Wz": np.ascontiguousarray(Wz, np.float32),
        "Wr": np.ascontiguousarray(Wr, np.float32),
        "Wh": np.ascontiguousarray(Wh, np.float32),
        "Uz": np.ascontiguousarray(Uz).astype(bf),
        "Ur": np.ascontiguousarray(Ur).astype(bf),
        "Uh": np.ascontiguousarray(Uh).astype(bf),
        "bz": np.ascontiguousarray(bz.reshape(KH, 128).T, np.float32),
        "br": np.ascontiguousarray(br.reshape(KH, 128).T, np.float32),
        "bh": np.ascontiguousarray(bh.reshape(KH, 128).T, np.float32),
        "ident": np.eye(128, dtype=bf),
    }
    maps = []
    for c in range(NCORES):
        xc = x[c * BL : (c + 1) * BL, :n_steps]          # [BL, t, D]
        xTc = np.ascontiguousarray(
            xc.transpose(2, 1, 0).reshape(D, n_steps * BL), np.float32
        )
        maps.append({**common, "xT": xTc})
    return maps


def _assemble(results, n_steps: int = T):
    outs = np.empty((B, n_steps, H), np.float32)
    for c, res in enumerate(results):
        yT = res["yT"]                                    # [128, t, KH*BL]
        outs[c * BL : (c + 1) * BL] = (
            yT.reshape(128, n_steps, KH, BL).transpose(3, 1, 2, 0).reshape(BL, n_steps, H)
        )
    hT = np.ascontiguousarray(outs[:, -1, :])
    return outs, hT


def kernel(x, Wz, Wr, Wh, Uz, Ur, Uh, bz, br, bh):
    x = np.asarray(x, np.float32)
    nc = _get_nc(T)
    maps = _in_maps(x, Wz, Wr, Wh, np.asarray(Uz), np.asarray(Ur), np.asarray(Uh),
                    np.asarray(bz), np.asarray(br), np.asarray(bh), T)
    res = run_bass_kernel_spmd(nc, maps, core_ids=list(range(NCORES)))
    return _assemble(res.results, T)


# revision 7
# speedup vs baseline: 1.1447x; 1.1447x over previous
"""Trainium2 Bass kernel for a GRU (B=64, T=512, D=256, H=512), 8-core data-parallel.

Strategy
--------
- Data-parallel over batch: 8 samples per NeuronCore, 8 cores.
- Everything on-chip lives in a transposed [H(partition-chunked), ..., B] layout so
  the recurrent matmuls (U stationary, h streaming) and the elementwise gate math
  share a layout: no per-step transposes.
- Phase 1 (per core): xz/xr/xh = x @ W* + b* computed as stream-bound fp32r matmuls
  into per-chunk DRAM scratch tiles (fine-grained deps let the recurrence overlap
  the tail of the precompute, keeping the PE HAM-warm), layout [128p, 4m, TC, 8b].
- Phase 2: 512 sequential GRU steps. Per step and gate, PSUM accumulates
  identity-matmul(xg_t) + sum_k U[k,m]^T @ h[k]; ScalarE applies sigmoid/tanh.
  The recurrent state is kept twice: a bf16 copy on the critical path feeding the
  next step's matmuls (tanh->mul->add in bf16, ~3 short DVE/ACT ops), and an fp32
  copy computed off the critical path for the output history.
- Host side: shards batch, pre-transposes x, casts U to bf16, and re-assembles the
  [B, T, H] output + final hT from the per-core [128, T, 32] outputs.
"""

import numpy as np
import ml_dtypes
from contextlib import ExitStack

import concourse.bass as bass
import concourse.bacc as bacc
import concourse.tile as tile
import concourse.mybir as mybir
from concourse.bass_utils import run_bass_kernel_spmd

B, T, D, H = 64, 512, 256, 512
NCORES = 8
BL = B // NCORES            # local batch per core = 8
KD = D // 128               # 2 contraction chunks for x @ W
KH = H // 128               # 4 chunks of the hidden dim
FB = KH * BL                # 32 free columns of a [h-chunked, batch] tile
TPRE = 8                    # precompute tiles over the (t, b) column axis
TC = 32                     # steps per recurrence chunk (xg staging + hist drain)

FP32 = mybir.dt.float32
FP32R = mybir.dt.float32r
BF16 = mybir.dt.bfloat16
AF = mybir.ActivationFunctionType


def _build(n_steps: int = T):
    """Build the single-core Bass program (SPMD across 8 cores)."""
    assert n_steps % TC == 0
    nc = bacc.Bacc("TRN2", target_bir_lowering=False, debug=False, num_devices=NCORES)

    xT = nc.dram_tensor("xT", [D, n_steps * BL], FP32R, kind="ExternalInput")
    Ws = {g: nc.dram_tensor(f"W{g}", [D, H], FP32R, kind="ExternalInput") for g in "zrh"}
    Us = {g: nc.dram_tensor(f"U{g}", [H, H], BF16, kind="ExternalInput") for g in "zrh"}
    bs = {g: nc.dram_tensor(f"b{g}", [128, KH], FP32, kind="ExternalInput") for g in "zrh"}
    ident = nc.dram_tensor("ident", [128, 128], BF16, kind="ExternalInput")
    yT = nc.dram_tensor("yT", [128, n_steps, FB], FP32, kind="ExternalOutput")

    tpre = max(1, n_steps * BL // 512)      # number of precompute tiles
    pre_cols = n_steps * BL // tpre         # columns per precompute tile
    pre_steps = n_steps // tpre             # time steps per precompute tile
    n_chunks = n_steps // TC
    chunks_per_pre = pre_steps // TC        # recurrence chunks per precompute tile

    with tile.TileContext(nc) as tc, ExitStack() as ctx:
        wpool = ctx.enter_context(tc.tile_pool(name="wpool", bufs=1))
        dram = ctx.enter_context(tc.tile_pool(name="dram", bufs=1, space="DRAM"))
        xt_pool = ctx.enter_context(tc.tile_pool(name="xtp", bufs=2))
        pre_ps = ctx.enter_context(tc.tile_pool(name="preps", bufs=2, space="PSUM"))
        pre_out = ctx.enter_context(tc.tile_pool(name="preout", bufs=3))
        hpool = ctx.enter_context(tc.tile_pool(name="hstate", bufs=3))
        gates_ps = ctx.enter_context(tc.tile_pool(name="gps", bufs=2, space="PSUM"))
        ew = ctx.enter_context(tc.tile_pool(name="ew", bufs=2))
        xg_pool = ctx.enter_context(tc.tile_pool(name="xgs", bufs=2))
        hist_pool = ctx.enter_context(tc.tile_pool(name="hist", bufs=2))

        # --- load weights -------------------------------------------------
        U_sb, W_sb, b_sb = {}, {}, {}
        for g in "zrh":
            u_t = wpool.tile([128, KH, H], BF16, tag=f"U{g}", name=f"U{g}_sb")
            nc.sync.dma_start(u_t[:], Us[g].rearrange("(k p) c -> p k c", p=128))
            U_sb[g] = u_t
            w_t = wpool.tile([128, KD, H], FP32R, tag=f"W{g}", name=f"W{g}_sb")
            nc.sync.dma_start(w_t[:], Ws[g].rearrange("(k p) c -> p k c", p=128))
            W_sb[g] = w_t
            bias_t = wpool.tile([128, KH], FP32, tag=f"b{g}", name=f"b{g}_sb")
            nc.sync.dma_start(bias_t[:], bs[g][:])
            b_sb[g] = bias_t
        I_sb = wpool.tile([128, 128], BF16, tag="ident", name="I_sb")
        nc.sync.dma_start(I_sb[:], ident[:])

        # Per-chunk DRAM scratch for the input projections: [p, m, TC, b] bf16.
        # One tile per (gate, chunk) keeps the dep graph fine-grained so the
        # recurrence can start while the precompute is still running.
        xg_dr = {
            g: [
                dram.tile([128, KH, TC, BL], BF16, tag=f"xgd{g}{c}", name=f"xgd{g}{c}")
                for c in range(n_chunks)
            ]
            for g in "zrh"
        }

        # --- phase 1: xg = x @ Wg + bg ------------------------------------
        xTr = xT.rearrange("(k p) n -> p k n", p=128)
        for tt in range(tpre):
            xt = xt_pool.tile([128, KD, pre_cols], FP32R, tag="xt", name="xt")
            nc.sync.dma_start(xt[:], xTr[:, :, tt * pre_cols : (tt + 1) * pre_cols])
            for g in "zrh":
                for m in range(KH):
                    ps = pre_ps.tile([128, pre_cols], FP32, tag="ps", name="ps")
                    for k in range(KD):
                        nc.tensor.matmul(
                            ps[:],
                            W_sb[g][:, k, m * 128 : (m + 1) * 128],
                            xt[:, k, :],
                            start=(k == 0),
                            stop=(k == KD - 1),
                        )
                    ob = pre_out.tile([128, pre_cols], BF16, tag="ob", name="ob")
                    nc.scalar.activation(
                        ob[:], ps[:], AF.Identity, bias=b_sb[g][:, m : m + 1]
                    )
                    obv = ob[:].rearrange("p (c t b) -> p c t b", c=chunks_per_pre, t=TC)
                    for cc in range(chunks_per_pre):
                        nc.sync.dma_start(
                            xg_dr[g][tt * chunks_per_pre + cc][:, m, :, :],
                            obv[:, cc, :, :],
                        )

        # --- phase 2: recurrence ------------------------------------------
        h_bf = hpool.tile([128, KH, BL], BF16, tag="hbf", name="h_bf")
        nc.vector.memset(h_bf[:], 0.0)
        h_fp_prev = None  # fp32 h of the previous step (a hist slice)

        for c in range(n_chunks):
            xg_sb = {}
            for g in "zrh":
                xg_t = xg_pool.tile(
                    [128, KH, TC, BL], BF16, tag=f"xg{g}", name=f"xg{g}_sb"
                )
                nc.sync.dma_start(xg_t[:], xg_dr[g][c][:])
                xg_sb[g] = xg_t
            hist = hist_pool.tile([128, TC, FB], FP32, tag="hist", name="hist")

            for tl in range(TC):
                first = c == 0 and tl == 0
                ps = {
                    g: gates_ps.tile([128, KH, BL], FP32, tag=f"ps{g}", name=f"ps{g}")
                    for g in "zrh"
                }
                # accumulate xg_t into PSUM via an identity matmul
                for g in "zrh":
                    nc.tensor.matmul(
                        ps[g][:], I_sb[:], xg_sb[g][:, :, tl, :],
                        start=True, stop=False,
                    )
                # r gate first (its output gates the h~ matmul)
                for k in range(KH):
                    for m in range(KH):
                        nc.tensor.matmul(
                            ps["r"][:, m, :],
                            U_sb["r"][:, k, m * 128 : (m + 1) * 128],
                            h_bf[:, k, :],
                            start=False,
                            stop=(k == KH - 1 and m == KH - 1),
                        )
                r_bf = ew.tile([128, KH, BL], BF16, tag="r", name="r_bf")
                nc.scalar.activation(r_bf[:], ps["r"][:], AF.Sigmoid)
                rh = ew.tile([128, KH, BL], BF16, tag="rh", name="rh")
                nc.vector.tensor_mul(rh[:], r_bf[:], h_bf[:])

                # z gate (keeps PE busy during sigmoid(r) / r*h)
                for k in range(KH):
                    for m in range(KH):
                        nc.tensor.matmul(
                            ps["z"][:, m, :],
                            U_sb["z"][:, k, m * 128 : (m + 1) * 128],
                            h_bf[:, k, :],
                            start=False,
                            stop=(k == KH - 1 and m == KH - 1),
                        )
                # candidate gate uses r*h
                for k in range(KH):
                    for m in range(KH):
                        nc.tensor.matmul(
                            ps["h"][:, m, :],
                            U_sb["h"][:, k, m * 128 : (m + 1) * 128],
                            rh[:, k, :],
                            start=False,
                            stop=(k == KH - 1 and m == KH - 1),
                        )

                # gate activations; *_bf feed the fast bf16 state path,
                # *_fp the accurate fp32 history
                z_bf = ew.tile([128, KH, BL], BF16, tag="zbf", name="z_bf")
                nc.scalar.activation(z_bf[:], ps["z"][:], AF.Sigmoid)
                zc_bf = ew.tile([128, KH, BL], BF16, tag="zcbf", name="zc_bf")
                nc.scalar.activation(zc_bf[:], ps["z"][:], AF.Sigmoid, scale=-1.0)
                z_fp = ew.tile([128, KH, BL], FP32, tag="zfp", name="z_fp")
                nc.scalar.activation(z_fp[:], ps["z"][:], AF.Sigmoid)
                zc_fp = ew.tile([128, KH, BL], FP32, tag="zcfp", name="zc_fp")
                nc.scalar.activation(zc_fp[:], ps["z"][:], AF.Sigmoid, scale=-1.0)
                ht_bf = ew.tile([128, KH, BL], BF16, tag="htbf", name="ht_bf")
                nc.scalar.activation(ht_bf[:], ps["h"][:], AF.Tanh)
                ht_fp = ew.tile([128, KH, BL], FP32, tag="htfp", name="ht_fp")
                nc.scalar.activation(ht_fp[:], ps["h"][:], AF.Tanh)

                h_new = hist[:, tl, :].rearrange("p (m b) -> p m b", m=KH)
                a_bf = ew.tile([128, KH, BL], BF16, tag="abf", name="a_bf")
                nc.vector.tensor_mul(a_bf[:], zc_bf[:], ht_bf[:])
                a_fp = ew.tile([128, KH, BL], FP32, tag="afp", name="a_fp")
                nc.vector.tensor_mul(a_fp[:], zc_fp[:], ht_fp[:])
                h_bf_next = hpool.tile([128, KH, BL], BF16, tag="hbf", name="h_bf")
                if first:
                    # h_prev = 0 so h' = (1-z)*h~
                    nc.vector.tensor_copy(h_bf_next[:], a_bf[:])
                    nc.vector.tensor_copy(h_new, a_fp[:])
                else:
                    zh_bf = ew.tile([128, KH, BL], BF16, tag="zhbf", name="zh_bf")
                    nc.vector.tensor_mul(zh_bf[:], z_bf[:], h_bf[:])
                    nc.vector.tensor_add(h_bf_next[:], a_bf[:], zh_bf[:])
                    zh_fp = ew.tile([128, KH, BL], FP32, tag="zhfp", name="zh_fp")
                    nc.vector.tensor_mul(zh_fp[:], z_fp[:], h_fp_prev)
                    nc.vector.tensor_add(h_new, a_fp[:], zh_fp[:])
                h_bf = h_bf_next
                h_fp_prev = h_new

            nc.sync.dma_start(yT[:, c * TC : (c + 1) * TC, :], hist[:])

    nc.compile()
    return nc


_NC_CACHE = {}


def _get_nc(n_steps: int = T):
    if n_steps not in _NC_CACHE:
        _NC_CACHE[n_steps] = _build(n_steps)
    return _NC_CACHE[n_steps]


def _in_maps(x, Wz, Wr, Wh, Uz, Ur, Uh, bz, br, bh, n_steps: int = T):
    bf = ml_dtypes.bfloat16
    common = {
        "Wz": np.ascontiguousarray(Wz, np.float32),
        "Wr": np.ascontiguousarray(Wr, np.float32),
        "Wh": np.ascontiguousarray(Wh, np.float32),
        "Uz": np.ascontiguousarray(Uz).astype(bf),
        "Ur": np.ascontiguousarray(Ur).astype(bf),
        "Uh": np.ascontiguousarray(Uh).astype(bf),
        "bz": np.ascontiguousarray(bz.reshape(KH, 128).T, np.float32),
        "br": np.ascontiguousarray(br.reshape(KH, 128).T, np.float32),
        "bh": np.ascontiguousarray(bh.reshape(KH, 128).T, np.float32),
        "ident": np.eye(128, dtype=bf),
    }
    maps = []
    for c in range(NCORES):
        xc = x[c * BL : (c + 1) * BL, :n_steps]          # [BL, t, D]
        xTc = np.ascontiguousarray(
            xc.transpose(2, 1, 0).reshape(D, n_steps * BL), np.float32
        )
        maps.append({**common, "xT": xTc})
    return maps


def _assemble(results, n_steps: int = T):
    outs = np.empty((B, n_steps, H), np.float32)
    for c, res in enumerate(results):
        yT = res["yT"]                                    # [128, t, KH*BL]
        outs[c * BL : (c + 1) * BL] = (
            yT.reshape(128, n_steps, KH, BL).transpose(3, 1, 2, 0).reshape(BL, n_steps, H)
        )
    hT = np.ascontiguousarray(outs[:, -1, :])
    return outs, hT


def kernel(x, Wz, Wr, Wh, Uz, Ur, Uh, bz, br, bh):
    x = np.asarray(x, np.float32)
    nc = _get_nc(T)
    maps = _in_maps(x, Wz, Wr, Wh, np.asarray(Uz), np.asarray(Ur), np.asarray(Uh),
                    np.asarray(bz), np.asarray(br), np.asarray(bh), T)
    res = run_bass_kernel_spmd(nc, maps, core_ids=list(range(NCORES)))
    return _assemble(res.results, T)
